# revision 11
# baseline (speedup 1.0000x reference)
"""Trainium2 Bass kernel for nn_Attention_46858093199829.

Math note (why x and b are never read on-device):
    score[b,i,j] = x[b,i] @ wx + key[j] @ wk + b0
The x-dependent term and the bias are constant in j, so they cancel in
softmax over j:
    softmax_j(score[b,i,:]) = softmax(key @ wk)          (same for every b,i)
    out[b,i,:]              = softmax(key @ wk) @ value  (a single 3-vector)

Sharding: data-parallel over batch B=32 -> 4 batches per core. key/value/W
are replicated; every core computes the (identical) 3-vector and writes its
own (4,512,3) output shard.

Implementation: raw Bacc (no TileContext) with manual semaphores on three
engines - sync issues all DMAs (HWDGE), vector does the elementwise work and
reductions, scalar does the exp. Tile's scheduling scaffolding (event-sem
vector clocks, per-engine drains, exit barrier butterfly) costs ~18us on a
kernel whose real work is ~3us, so it is written out by hand instead.

Cross-partition tricks (validated on HW):
  - reduce across partitions: DMA-collapse pcat (128,4) -> sfold (1,512),
    then a strided free-axis reduce on one partition.
  - broadcast result to the (2048,3) output: bounce v* (1,48) through a
    DRAM scratch, then one DRAM->DRAM DMA with a 0-step partition source
    (SBUF sources cannot have 0-step partition dims; DRAM sources can).
"""

import sys
import types

import numpy as np

import concourse.bacc as bacc
import concourse.bass as bass
from concourse import mybir
from concourse.bass_utils import run_bass_kernel_spmd


def _install_ntff_shim():
    """bass_utils' trace path imports antenv.axon_hooks, which some images
    lack. Provide it, backed by trn_boot's ctypes NTFF hook when available;
    returning None makes bass_utils skip tracing instead of crashing."""
    if "antenv.axon_hooks" in sys.modules:
        return
    try:
        import antenv

        m = types.ModuleType("antenv.axon_hooks")
        _state = {"hook": None, "tried": False}

        def set_axon_ntff_profile_hook(h):
            _state["hook"] = h
            _state["tried"] = True

        def get_axon_ntff_profile_hook():
            if not _state["tried"]:
                _state["tried"] = True
                try:
                    from trn_agent_boot.trn_boot import _ntff_profile_via_ctypes

                    _state["hook"] = _ntff_profile_via_ctypes(
                        "/opt/axon/libaxon_pjrt.so"
                    )
                except Exception:
                    _state["hook"] = None
            return _state["hook"]

        m.set_axon_ntff_profile_hook = set_axon_ntff_profile_hook
        m.get_axon_ntff_profile_hook = get_axon_ntff_profile_hook
        sys.modules["antenv.axon_hooks"] = m
        antenv.axon_hooks = m
    except Exception:
        pass


_install_ntff_shim()

N_CORES = 8
B, S1, S2, D = 32, 512, 2048, 3
P, NF = 128, 16  # 2048 = 128 partitions x 16 free
ROWS_PER_CORE = (B // N_CORES) * S1  # 2048 rows of (3,) per core

# Exposed for the test harness: the BassKernelResults of the last run
# (carries exec_time_ns when BASS_TRACE=1).
last_result = None

_nc_cache = None


def _build():
    nc = bacc.Bacc(target_bir_lowering=False, debug=False)
    f32 = mybir.dt.float32
    key_t = nc.dram_tensor("key", [S2, D], f32, kind="ExternalInput")
    val_t = nc.dram_tensor("value", [S2, D], f32, kind="ExternalInput")
    w_t = nc.dram_tensor("W", [1, 6], f32, kind="ExternalInput")
    out_t = nc.dram_tensor("out", [ROWS_PER_CORE, D], f32, kind="ExternalOutput")
    scratch = nc.dram_tensor("vstar_scratch", [1, NF * D], f32)

    from contextlib import ExitStack

    with ExitStack() as ctx:
        ec = ctx.enter_context
        kv = ec(nc.sbuf_tensor("kv", [P, NF, D], f32))
        vv = ec(nc.sbuf_tensor("vv", [P, NF, D], f32))
        wb = ec(nc.sbuf_tensor("wb", [P, 6], f32))
        t0 = ec(nc.sbuf_tensor("t0", [P, NF], f32))
        t1 = ec(nc.sbuf_tensor("t1", [P, NF], f32))
        t2 = ec(nc.sbuf_tensor("t2", [P, NF], f32))
        u01 = ec(nc.sbuf_tensor("u01", [P, NF], f32))
        sk = ec(nc.sbuf_tensor("sk", [P, NF], f32))
        e = ec(nc.sbuf_tensor("e", [P, NF], f32))
        esum = ec(nc.sbuf_tensor("esum", [P, 1], f32))
        bz = ec(nc.sbuf_tensor("bz", [P, 1], f32))
        m0 = ec(nc.sbuf_tensor("m0", [P, NF], f32))
        m1 = ec(nc.sbuf_tensor("m1", [P, NF], f32))
        m2 = ec(nc.sbuf_tensor("m2", [P, NF], f32))
        pcat = ec(nc.sbuf_tensor("pcat", [P, 4], f32))
        sfold = ec(nc.sbuf_tensor("sfold", [1, P * 4], f32))
        red4 = ec(nc.sbuf_tensor("red4", [1, 4], f32))
        rz = ec(nc.sbuf_tensor("rz", [1, 1], f32))
        vst = ec(nc.sbuf_tensor("vst", [1, D], f32))
        dL = ec(nc.semaphore("dL"))      # input loads
        dC = ec(nc.semaphore("dC"))      # pcat collapse
        dS = ec(nc.semaphore("dS"))      # v* -> DRAM scratch
        dO = ec(nc.semaphore("dO"))      # final broadcast store
        vp = ec(nc.semaphore("vp"))      # vector same-engine pipeline sem
        s_sk = ec(nc.semaphore("s_sk"))  # vector -> scalar: sk ready
        s_e = ec(nc.semaphore("s_e"))    # scalar -> vector: e/esum ready
        s_pc = ec(nc.semaphore("s_pc"))  # vector -> sync: pcat ready (x4)
        s_v = ec(nc.semaphore("s_v"))    # vector -> sync: v* ready
        block = ec(nc.Block())

        @block.sync
        def _(eng):
            eng.dma_start(
                out=kv[:, :, :], in_=key_t[:, :].rearrange("(p n) d -> p n d", p=P)
            ).then_inc(dL, 16)
            eng.dma_start(
                out=vv[:, :, :], in_=val_t[:, :].rearrange("(p n) d -> p n d", p=P)
            ).then_inc(dL, 16)
            eng.dma_start(out=wb[:, :], in_=w_t[:, :].to_broadcast([P, 6])).then_inc(
                dL, 16
            )
            # collapse pcat (128,4) onto one partition: sfold[0, i*4+j] = pcat[i, j]
            eng.wait_ge(s_pc, 4)
            eng.dma_start(out=sfold[0:1, :], in_=pcat[:, :]).then_inc(dC, 16)
            # bounce v* through DRAM (replicated x16), then broadcast-store to
            # the full output with a 0-step partition source (DRAM allows it)
            eng.wait_ge(s_v, 1)
            vst_rep = bass.AP(
                tensor=vst.ap().tensor, offset=0, ap=[[1, 1], [0, NF], [1, D]]
            )
            eng.dma_start(out=scratch[:, :], in_=vst_rep).then_inc(dS, 16)
            eng.wait_ge(dS, 16)
            eng.dma_start(
                out=out_t[:, :].rearrange("(p n) d -> p (n d)", p=P),
                in_=scratch[:, :].to_broadcast([P, NF * D]),
            ).then_inc(dO, 16)
            eng.wait_ge(dO, 16)

        @block.vector
        def _(eng):
            nc.vector.memset(bz[:, :], 0.0)
            # sk[p,n] = sum_d key[p,n,d] * wk[d]   (wk = W[0, 3:6])
            eng.wait_ge(dL, 48)
            nc.vector.tensor_scalar_mul(t0[:, :], kv[:, :, 0], wb[:, 3:4]).then_inc(vp)
            nc.vector.tensor_scalar_mul(t1[:, :], kv[:, :, 1], wb[:, 4:5]).then_inc(vp)
            nc.vector.tensor_scalar_mul(t2[:, :], kv[:, :, 2], wb[:, 5:6]).then_inc(vp)
            eng.wait_ge(vp, 3)
            nc.vector.tensor_add(u01[:, :], t0[:, :], t1[:, :]).then_inc(vp)
            eng.wait_ge(vp, 4)
            nc.vector.tensor_add(sk[:, :], u01[:, :], t2[:, :]).then_inc(s_sk, 1)
            # pcat[:,d] = per-partition sum of e * value[:,:,d]; pcat[:,3] = esum
            eng.wait_ge(s_e, 1)
            nc.vector.tensor_mul(m0[:, :], e[:, :], vv[:, :, 0]).then_inc(vp)
            nc.vector.tensor_mul(m1[:, :], e[:, :], vv[:, :, 1]).then_inc(vp)
            nc.vector.tensor_mul(m2[:, :], e[:, :], vv[:, :, 2]).then_inc(vp)
            eng.wait_ge(vp, 7)
            for d, m in enumerate((m0, m1, m2)):
                nc.vector.reduce_sum(
                    pcat[:, d : d + 1], m[:, :], axis=mybir.AxisListType.X
                ).then_inc(s_pc, 1)
            nc.vector.tensor_copy(pcat[:, 3:4], esum[:, :]).then_inc(s_pc, 1)
            # final reduction over the collapsed 128 partials, then v* = num/Z
            eng.wait_ge(dC, 16)
            nc.vector.reduce_sum(
                red4[0:1, :],
                sfold[0:1, :].rearrange("p (i j) -> p j i", j=4),
                axis=mybir.AxisListType.X,
            ).then_inc(vp)
            eng.wait_ge(vp, 8)
            nc.vector.reciprocal(rz[0:1, :], red4[0:1, 3:4]).then_inc(vp)
            eng.wait_ge(vp, 9)
            nc.vector.tensor_scalar_mul(
                vst[0:1, :], red4[0:1, 0:3], rz[0:1, 0:1]
            ).then_inc(s_v, 1)

        @block.scalar
        def _(eng):
            eng.wait_ge(s_sk, 1)
            nc.scalar.activation(
                e[:, :],
                sk[:, :],
                mybir.ActivationFunctionType.Exp,
                bias=bz[:, 0:1],
                accum_out=esum[:, :],
            ).then_inc(s_e, 1)

    nc.compile()
    return nc


def kernel(x, key, value, W, b):
    global last_result, _nc_cache
    key = np.ascontiguousarray(np.asarray(key, dtype=np.float32))
    value = np.ascontiguousarray(np.asarray(value, dtype=np.float32))
    W = np.ascontiguousarray(np.asarray(W, dtype=np.float32))
    if _nc_cache is None:
        _nc_cache = _build()
    in_maps = [
        {"key": key, "value": value, "W": W} for _ in range(N_CORES)
    ]
    res = run_bass_kernel_spmd(_nc_cache, in_maps, core_ids=list(range(N_CORES)))
    last_result = res
    out = np.concatenate([r["out"] for r in res.results], axis=0)
    return out.reshape(B, S1, D)


# revision 13
# speedup vs baseline: 1.6244x; 1.6244x over previous
"""Trainium2 Bass kernel for nn_Attention_46858093199829.

Math note (why x and b are never read on-device):
    score[b,i,j] = x[b,i] @ wx + key[j] @ wk + b0
The x-dependent term and the bias are constant in j, so they cancel in
softmax over j:
    softmax_j(score[b,i,:]) = softmax(key @ wk)          (same for every b,i)
    out[b,i,:]              = softmax(key @ wk) @ value  (a single 3-vector)

Sharding: data-parallel over batch B=32 -> 4 batches per core. key/value/W
are replicated; every core computes the (identical) 3-vector and writes its
own (4,512,3) output shard.

Implementation: raw Bacc (no TileContext) with manual semaphores on three
engines - sync issues all DMAs (HWDGE), vector does the elementwise work and
reductions, scalar does the exp. Tile's scheduling scaffolding (event-sem
vector clocks, per-engine drains, exit barrier butterfly) costs ~18us on a
kernel whose real work is ~3us, so it is written out by hand instead.

Cross-partition tricks (validated on HW):
  - reduce across partitions: DMA-collapse pcat (128,4) -> sfold (1,512),
    then a strided free-axis reduce on one partition.
  - broadcast result to the (2048,3) output: bounce v* (1,48) through a
    DRAM scratch, then one DRAM->DRAM DMA with a 0-step partition source
    (SBUF sources cannot have 0-step partition dims; DRAM sources can).
"""

import sys
import types

import numpy as np

import concourse.bacc as bacc
import concourse.bass as bass
from concourse import mybir
from concourse.bass_utils import run_bass_kernel_spmd


def _install_ntff_shim():
    """bass_utils' trace path imports antenv.axon_hooks, which some images
    lack. Provide it, backed by trn_boot's ctypes NTFF hook when available;
    returning None makes bass_utils skip tracing instead of crashing."""
    if "antenv.axon_hooks" in sys.modules:
        return
    try:
        import antenv

        m = types.ModuleType("antenv.axon_hooks")
        _state = {"hook": None, "tried": False}

        def set_axon_ntff_profile_hook(h):
            _state["hook"] = h
            _state["tried"] = True

        def get_axon_ntff_profile_hook():
            if not _state["tried"]:
                _state["tried"] = True
                try:
                    from trn_agent_boot.trn_boot import _ntff_profile_via_ctypes

                    _state["hook"] = _ntff_profile_via_ctypes(
                        "/opt/axon/libaxon_pjrt.so"
                    )
                except Exception:
                    _state["hook"] = None
            return _state["hook"]

        m.set_axon_ntff_profile_hook = set_axon_ntff_profile_hook
        m.get_axon_ntff_profile_hook = get_axon_ntff_profile_hook
        sys.modules["antenv.axon_hooks"] = m
        antenv.axon_hooks = m
    except Exception:
        pass


_install_ntff_shim()

N_CORES = 8
B, S1, S2, D = 32, 512, 2048, 3
P, NF = 128, 16  # 2048 = 128 partitions x 16 free
ROWS_PER_CORE = (B // N_CORES) * S1  # 2048 rows of (3,) per core

# Exposed for the test harness: the BassKernelResults of the last run
# (carries exec_time_ns when BASS_TRACE=1).
last_result = None

_nc_cache = None


def _strip_framework_overhead(nc):
    """Remove Bass' hardcoded framework scaffolding: the const-AP memsets
    (nothing here reads them) and the entry/exit all-engine barriers
    (Drain + barrier_* EventSemaphores). All ordering in this kernel is
    carried by its own semaphores; the runtime resets semaphores per
    execution, so no cross-engine barrier is needed at entry or exit."""

    def is_barrier_sync(si):
        if si is None:
            return False
        refs = [w.ant_name for w in (si.on_wait or [])] + [
            u.ant_name for u in (si.on_update or [])
        ]
        return bool(refs) and all(r.startswith("barrier_") for r in refs)

    for f in nc.m.functions:
        for bb in f.blocks:
            kept = []
            for inst in bb.instructions:
                t = type(inst).__name__
                if t == "InstDrain":
                    continue
                if (
                    t == "InstMemset"
                    and inst.outs
                    and getattr(inst.outs[0], "memref", "").startswith("const-")
                ):
                    continue
                if t == "InstEventSemaphore" and is_barrier_sync(inst.sync_info):
                    continue
                kept.append(inst)
            if len(kept) != len(bb.instructions):
                bb.instructions[:] = kept


def _build():
    nc = bacc.Bacc(target_bir_lowering=False, debug=False)
    f32 = mybir.dt.float32
    key_t = nc.dram_tensor("key", [S2, D], f32, kind="ExternalInput")
    val_t = nc.dram_tensor("value", [S2, D], f32, kind="ExternalInput")
    w_t = nc.dram_tensor("W", [1, 6], f32, kind="ExternalInput")
    out_t = nc.dram_tensor("out", [ROWS_PER_CORE, D], f32, kind="ExternalOutput")

    from contextlib import ExitStack

    with ExitStack() as ctx:
        ec = ctx.enter_context
        kv = ec(nc.sbuf_tensor("kv", [P, NF, D], f32))
        vv = ec(nc.sbuf_tensor("vv", [P, NF, D], f32))
        wb = ec(nc.sbuf_tensor("wb", [P, 6], f32))
        t0 = ec(nc.sbuf_tensor("t0", [P, NF], f32))
        t1 = ec(nc.sbuf_tensor("t1", [P, NF], f32))
        t2 = ec(nc.sbuf_tensor("t2", [P, NF], f32))
        u01 = ec(nc.sbuf_tensor("u01", [P, NF], f32))
        sk = ec(nc.sbuf_tensor("sk", [P, NF], f32))
        e = ec(nc.sbuf_tensor("e", [P, NF], f32))
        esum = ec(nc.sbuf_tensor("esum", [P, 1], f32))
        bz = ec(nc.sbuf_tensor("bz", [P, 1], f32))
        m0 = ec(nc.sbuf_tensor("m0", [P, NF], f32))
        m1 = ec(nc.sbuf_tensor("m1", [P, NF], f32))
        m2 = ec(nc.sbuf_tensor("m2", [P, NF], f32))
        pcat = ec(nc.sbuf_tensor("pcat", [P, 4], f32))
        ones = ec(nc.sbuf_tensor("ones", [P, P], f32))
        rz = ec(nc.sbuf_tensor("rz", [P, 1], f32))
        ot = ec(nc.sbuf_tensor("ot", [P, NF, D], f32))
        redp = ec(nc.psum_tensor("redp", [P, 4], f32))
        dW = ec(nc.semaphore("dW"))      # W broadcast load
        dK = ec(nc.semaphore("dK"))      # key load
        dV = ec(nc.semaphore("dV"))      # value load
        dO = ec(nc.semaphore("dO"))      # final store
        vp = ec(nc.semaphore("vp"))      # vector same-engine pipeline sem
        s_sk = ec(nc.semaphore("s_sk"))  # vector -> scalar: sk ready
        s_e = ec(nc.semaphore("s_e"))    # scalar -> vector: e/esum ready
        s_pc = ec(nc.semaphore("s_pc"))  # vector -> tensor: pcat ready (x4)
        s_mm = ec(nc.semaphore("s_mm"))  # tensor -> vector: reduction done
        s_ot = ec(nc.semaphore("s_ot"))  # vector -> sync: out tile ready
        block = ec(nc.Block())

        @block.sync
        def _(eng):
            eng.dma_start(out=wb[:, :], in_=w_t[:, :].to_broadcast([P, 6])).then_inc(
                dW, 16
            )
            eng.dma_start(
                out=kv[:, :, :], in_=key_t[:, :].rearrange("(p n) d -> p n d", p=P)
            ).then_inc(dK, 16)
            eng.dma_start(
                out=vv[:, :, :], in_=val_t[:, :].rearrange("(p n) d -> p n d", p=P)
            ).then_inc(dV, 16)
            eng.wait_ge(s_ot, 1)
            eng.dma_start(
                out=out_t[:, :].rearrange("(p n) d -> p n d", p=P), in_=ot[:, :, :]
            ).then_inc(dO, 16)
            eng.wait_ge(dO, 16)

        @block.vector
        def _(eng):
            nc.vector.memset(bz[:, :], 0.0)
            nc.vector.memset(ones[:, :], 1.0)
            # sk[p,n] = sum_d key[p,n,d] * wk[d]   (wk = W[0, 3:6])
            eng.wait_ge(dW, 16)
            eng.wait_ge(dK, 16)
            nc.vector.tensor_scalar_mul(t0[:, :], kv[:, :, 0], wb[:, 3:4]).then_inc(vp)
            nc.vector.tensor_scalar_mul(t1[:, :], kv[:, :, 1], wb[:, 4:5]).then_inc(vp)
            nc.vector.tensor_scalar_mul(t2[:, :], kv[:, :, 2], wb[:, 5:6]).then_inc(vp)
            eng.wait_ge(vp, 3)
            nc.vector.tensor_add(u01[:, :], t0[:, :], t1[:, :]).then_inc(vp)
            eng.wait_ge(vp, 4)
            nc.vector.tensor_add(sk[:, :], u01[:, :], t2[:, :]).then_inc(s_sk, 1)
            # pcat[:,d] = per-partition sum of e * value[:,:,d]; pcat[:,3] = esum
            eng.wait_ge(s_e, 1)
            eng.wait_ge(dV, 16)
            nc.vector.tensor_mul(m0[:, :], e[:, :], vv[:, :, 0]).then_inc(vp)
            nc.vector.tensor_mul(m1[:, :], e[:, :], vv[:, :, 1]).then_inc(vp)
            nc.vector.tensor_mul(m2[:, :], e[:, :], vv[:, :, 2]).then_inc(vp)
            eng.wait_ge(vp, 7)
            for d, m in enumerate((m0, m1, m2)):
                nc.vector.reduce_sum(
                    pcat[:, d : d + 1], m[:, :], axis=mybir.AxisListType.X
                ).then_inc(s_pc, 1)
            nc.vector.tensor_copy(pcat[:, 3:4], esum[:, :]).then_inc(s_pc, 1)
            # v* = num/Z straight out of PSUM, broadcast x16 into the out tile
            # via a 0-step middle dim on the input AP
            eng.wait_ge(s_mm, 1)
            nc.vector.reciprocal(rz[:, :], redp.ap()[:, 3:4]).then_inc(vp)
            eng.wait_ge(vp, 8)
            num_b = bass.AP(
                tensor=redp.ap().tensor, offset=0, ap=[[4, P], [0, NF], [1, D]]
            )
            nc.vector.tensor_scalar_mul(ot[:, :, :], num_b, rz[:, 0:1]).then_inc(
                s_ot, 1
            )

        @block.scalar
        def _(eng):
            eng.wait_ge(s_sk, 1)
            nc.scalar.activation(
                e[:, :],
                sk[:, :],
                mybir.ActivationFunctionType.Exp,
                bias=bz[:, 0:1],
                accum_out=esum[:, :],
            ).then_inc(s_e, 1)

        @block.tensor
        def _(eng):
            # ones(128,128).T @ pcat(128,4): column sums replicated on all
            # 128 partitions - cross-partition reduce AND broadcast in one op
            eng.wait_ge(s_pc, 4)
            nc.tensor.matmul(
                redp.ap()[:, :], ones[:, :], pcat[:, :], start=True, stop=True
            ).then_inc(s_mm, 1)

    _strip_framework_overhead(nc)
    nc.compile()
    return nc


def kernel(x, key, value, W, b):
    global last_result, _nc_cache
    key = np.ascontiguousarray(np.asarray(key, dtype=np.float32))
    value = np.ascontiguousarray(np.asarray(value, dtype=np.float32))
    W = np.ascontiguousarray(np.asarray(W, dtype=np.float32))
    if _nc_cache is None:
        _nc_cache = _build()
    in_maps = [
        {"key": key, "value": value, "W": W} for _ in range(N_CORES)
    ]
    res = run_bass_kernel_spmd(_nc_cache, in_maps, core_ids=list(range(N_CORES)))
    last_result = res
    out = np.concatenate([r["out"] for r in res.results], axis=0)
    return out.reshape(B, S1, D)


# revision 14
# speedup vs baseline: 1.7007x; 1.0470x over previous
"""Trainium2 Bass kernel for nn_Attention_46858093199829.

Math note (why x and b are never read on-device):
    score[b,i,j] = x[b,i] @ wx + key[j] @ wk + b0
The x-dependent term and the bias are constant in j, so they cancel in
softmax over j:
    softmax_j(score[b,i,:]) = softmax(key @ wk)          (same for every b,i)
    out[b,i,:]              = softmax(key @ wk) @ value  (a single 3-vector)

Sharding: data-parallel over batch B=32 -> 4 batches per core. key/value/W
are replicated; every core computes the (identical) 3-vector and writes its
own (4,512,3) output shard.

Implementation: raw Bacc (no TileContext) with manual semaphores on three
engines - sync issues all DMAs (HWDGE), vector does the elementwise work and
reductions, scalar does the exp. Tile's scheduling scaffolding (event-sem
vector clocks, per-engine drains, exit barrier butterfly) costs ~18us on a
kernel whose real work is ~3us, so it is written out by hand instead.

Cross-partition tricks (validated on HW):
  - reduce across partitions: DMA-collapse pcat (128,4) -> sfold (1,512),
    then a strided free-axis reduce on one partition.
  - broadcast result to the (2048,3) output: bounce v* (1,48) through a
    DRAM scratch, then one DRAM->DRAM DMA with a 0-step partition source
    (SBUF sources cannot have 0-step partition dims; DRAM sources can).
"""

import sys
import types

import numpy as np

import concourse.bacc as bacc
import concourse.bass as bass
from concourse import mybir
from concourse.bass_utils import run_bass_kernel_spmd


def _install_ntff_shim():
    """bass_utils' trace path imports antenv.axon_hooks, which some images
    lack. Provide it, backed by trn_boot's ctypes NTFF hook when available;
    returning None makes bass_utils skip tracing instead of crashing."""
    if "antenv.axon_hooks" in sys.modules:
        return
    try:
        import antenv

        m = types.ModuleType("antenv.axon_hooks")
        _state = {"hook": None, "tried": False}

        def set_axon_ntff_profile_hook(h):
            _state["hook"] = h
            _state["tried"] = True

        def get_axon_ntff_profile_hook():
            if not _state["tried"]:
                _state["tried"] = True
                try:
                    from trn_agent_boot.trn_boot import _ntff_profile_via_ctypes

                    _state["hook"] = _ntff_profile_via_ctypes(
                        "/opt/axon/libaxon_pjrt.so"
                    )
                except Exception:
                    _state["hook"] = None
            return _state["hook"]

        m.set_axon_ntff_profile_hook = set_axon_ntff_profile_hook
        m.get_axon_ntff_profile_hook = get_axon_ntff_profile_hook
        sys.modules["antenv.axon_hooks"] = m
        antenv.axon_hooks = m
    except Exception:
        pass


_install_ntff_shim()

N_CORES = 8
B, S1, S2, D = 32, 512, 2048, 3
P, NF = 128, 16  # 2048 = 128 partitions x 16 free
ROWS_PER_CORE = (B // N_CORES) * S1  # 2048 rows of (3,) per core

# Exposed for the test harness: the BassKernelResults of the last run
# (carries exec_time_ns when BASS_TRACE=1).
last_result = None

_nc_cache = None


def _strip_framework_overhead(nc):
    """Remove Bass' hardcoded framework scaffolding: the const-AP memsets
    (nothing here reads them) and the entry/exit all-engine barriers
    (Drain + barrier_* EventSemaphores). All ordering in this kernel is
    carried by its own semaphores; the runtime resets semaphores per
    execution, so no cross-engine barrier is needed at entry or exit."""

    def is_barrier_sync(si):
        if si is None:
            return False
        refs = [w.ant_name for w in (si.on_wait or [])] + [
            u.ant_name for u in (si.on_update or [])
        ]
        return bool(refs) and all(r.startswith("barrier_") for r in refs)

    for f in nc.m.functions:
        for bb in f.blocks:
            kept = []
            for inst in bb.instructions:
                t = type(inst).__name__
                if t == "InstDrain":
                    continue
                if (
                    t == "InstMemset"
                    and inst.outs
                    and getattr(inst.outs[0], "memref", "").startswith("const-")
                ):
                    continue
                if t == "InstEventSemaphore" and is_barrier_sync(inst.sync_info):
                    continue
                kept.append(inst)
            if len(kept) != len(bb.instructions):
                bb.instructions[:] = kept


def _build():
    nc = bacc.Bacc(target_bir_lowering=False, debug=False)
    f32 = mybir.dt.float32
    key_t = nc.dram_tensor("key", [S2, D], f32, kind="ExternalInput")
    val_t = nc.dram_tensor("value", [S2, D], f32, kind="ExternalInput")
    w_t = nc.dram_tensor("W", [1, 6], f32, kind="ExternalInput")
    out_t = nc.dram_tensor("out", [ROWS_PER_CORE, D], f32, kind="ExternalOutput")

    from contextlib import ExitStack

    # 32 partitions x 64 rows: 4x fewer DMA descriptors per transfer than a
    # 128-partition layout; DVE/ACT op time is overhead-dominated either way.
    Q, QF = 32, 64  # 2048 = 32 * 64

    with ExitStack() as ctx:
        ec = ctx.enter_context
        kv = ec(nc.sbuf_tensor("kv", [Q, QF, D], f32))
        vv = ec(nc.sbuf_tensor("vv", [Q, QF, D], f32))
        wb = ec(nc.sbuf_tensor("wb", [Q, 6], f32))
        t0 = ec(nc.sbuf_tensor("t0", [Q, QF], f32))
        t1 = ec(nc.sbuf_tensor("t1", [Q, QF], f32))
        t2 = ec(nc.sbuf_tensor("t2", [Q, QF], f32))
        u01 = ec(nc.sbuf_tensor("u01", [Q, QF], f32))
        sk = ec(nc.sbuf_tensor("sk", [Q, QF], f32))
        e = ec(nc.sbuf_tensor("e", [Q, QF], f32))
        esum = ec(nc.sbuf_tensor("esum", [Q, 1], f32))
        bz = ec(nc.sbuf_tensor("bz", [Q, 1], f32))
        m0 = ec(nc.sbuf_tensor("m0", [Q, QF], f32))
        m1 = ec(nc.sbuf_tensor("m1", [Q, QF], f32))
        m2 = ec(nc.sbuf_tensor("m2", [Q, QF], f32))
        pcat = ec(nc.sbuf_tensor("pcat", [Q, 4], f32))
        ones = ec(nc.sbuf_tensor("ones", [Q, Q], f32))
        rz = ec(nc.sbuf_tensor("rz", [Q, 1], f32))
        ot = ec(nc.sbuf_tensor("ot", [Q, QF, D], f32))
        redp = ec(nc.psum_tensor("redp", [Q, 4], f32))
        dW = ec(nc.semaphore("dW"))      # W broadcast load
        dK = ec(nc.semaphore("dK"))      # key load
        dV = ec(nc.semaphore("dV"))      # value load
        dO = ec(nc.semaphore("dO"))      # final store
        vp = ec(nc.semaphore("vp"))      # vector same-engine pipeline sem
        s_sk = ec(nc.semaphore("s_sk"))  # vector -> scalar: sk ready
        s_e = ec(nc.semaphore("s_e"))    # scalar -> vector: e/esum ready
        s_pc = ec(nc.semaphore("s_pc"))  # vector -> tensor: pcat ready (x4)
        s_mm = ec(nc.semaphore("s_mm"))  # tensor -> vector: reduction done
        s_ot = ec(nc.semaphore("s_ot"))  # vector -> sync: out tile ready

        # No nc.Block(): straight-line single-bb program, engines' streams
        # interleave by engine tag; ordering is purely semaphore-driven.

        # --- sync (SP) queue: W + key loads, final store ---
        nc.sync.dma_start(out=wb[:, :], in_=w_t[:, :].to_broadcast([Q, 6])).then_inc(
            dW, 16
        )
        nc.sync.dma_start(
            out=kv[:, :, :], in_=key_t[:, :].rearrange("(p n) d -> p n d", p=Q)
        ).then_inc(dK, 16)

        # --- scalar (ACT) queue: value load in parallel, then exp ---
        nc.scalar.dma_start(
            out=vv[:, :, :], in_=val_t[:, :].rearrange("(p n) d -> p n d", p=Q)
        ).then_inc(dV, 16)

        # --- vector queue ---
        nc.vector.memset(bz[:, :], 0.0)
        nc.vector.memset(ones[:, :], 1.0)
        # sk[p,n] = sum_d key[p,n,d] * wk[d]   (wk = W[0, 3:6])
        nc.vector.wait_ge(dW, 16)
        nc.vector.wait_ge(dK, 16)
        nc.vector.tensor_scalar_mul(t0[:, :], kv[:, :, 0], wb[:, 3:4]).then_inc(vp)
        nc.vector.tensor_scalar_mul(t1[:, :], kv[:, :, 1], wb[:, 4:5]).then_inc(vp)
        nc.vector.tensor_scalar_mul(t2[:, :], kv[:, :, 2], wb[:, 5:6]).then_inc(vp)
        nc.vector.wait_ge(vp, 3)
        nc.vector.tensor_add(u01[:, :], t0[:, :], t1[:, :]).then_inc(vp)
        nc.vector.wait_ge(vp, 4)
        nc.vector.tensor_add(sk[:, :], u01[:, :], t2[:, :]).then_inc(s_sk, 1)

        # --- scalar: e = exp(sk), esum = per-partition sum ---
        nc.scalar.wait_ge(s_sk, 1)
        nc.scalar.activation(
            e[:, :],
            sk[:, :],
            mybir.ActivationFunctionType.Exp,
            bias=bz[:, 0:1],
            accum_out=esum[:, :],
        ).then_inc(s_e, 1)

        # --- vector: pcat[:,d] = per-partition sum of e * value_d; +esum ---
        nc.vector.wait_ge(s_e, 1)
        nc.vector.wait_ge(dV, 16)
        nc.vector.tensor_mul(m0[:, :], e[:, :], vv[:, :, 0]).then_inc(vp)
        nc.vector.tensor_mul(m1[:, :], e[:, :], vv[:, :, 1]).then_inc(vp)
        nc.vector.tensor_mul(m2[:, :], e[:, :], vv[:, :, 2]).then_inc(vp)
        nc.vector.wait_ge(vp, 7)
        for d, m in enumerate((m0, m1, m2)):
            nc.vector.reduce_sum(
                pcat[:, d : d + 1], m[:, :], axis=mybir.AxisListType.X
            ).then_inc(s_pc, 1)
        nc.vector.tensor_copy(pcat[:, 3:4], esum[:, :]).then_inc(s_pc, 1)

        # --- tensor: ones(32,32).T @ pcat(32,4) reduces across partitions
        # AND broadcasts the 4 column sums to all 32 partitions in one op ---
        nc.tensor.wait_ge(s_pc, 4)
        nc.tensor.matmul(
            redp.ap()[:, :], ones[:, :], pcat[:, :], start=True, stop=True
        ).then_inc(s_mm, 1)

        # --- vector: v* = num/Z straight out of PSUM, broadcast x64 into the
        # out tile via a 0-step middle dim on the input AP ---
        nc.vector.wait_ge(s_mm, 1)
        nc.vector.reciprocal(rz[:, :], redp.ap()[:, 3:4]).then_inc(vp)
        nc.vector.wait_ge(vp, 8)
        num_b = bass.AP(
            tensor=redp.ap().tensor, offset=0, ap=[[4, Q], [0, QF], [1, D]]
        )
        nc.vector.tensor_scalar_mul(ot[:, :, :], num_b, rz[:, 0:1]).then_inc(s_ot, 1)

        # --- sync: store the full shard ---
        nc.sync.wait_ge(s_ot, 1)
        nc.sync.dma_start(
            out=out_t[:, :].rearrange("(p n) d -> p n d", p=Q), in_=ot[:, :, :]
        ).then_inc(dO, 16)
        nc.sync.wait_ge(dO, 16)

    _strip_framework_overhead(nc)
    nc.compile()
    return nc


def kernel(x, key, value, W, b):
    global last_result, _nc_cache
    key = np.ascontiguousarray(np.asarray(key, dtype=np.float32))
    value = np.ascontiguousarray(np.asarray(value, dtype=np.float32))
    W = np.ascontiguousarray(np.asarray(W, dtype=np.float32))
    if _nc_cache is None:
        _nc_cache = _build()
    in_maps = [
        {"key": key, "value": value, "W": W} for _ in range(N_CORES)
    ]
    res = run_bass_kernel_spmd(_nc_cache, in_maps, core_ids=list(range(N_CORES)))
    last_result = res
    out = np.concatenate([r["out"] for r in res.results], axis=0)
    return out.reshape(B, S1, D)


# revision 15
# speedup vs baseline: 1.7797x; 1.0464x over previous
"""Trainium2 Bass kernel for nn_Attention_46858093199829.

Math note (why x and b are never read on-device):
    score[b,i,j] = x[b,i] @ wx + key[j] @ wk + b0
The x-dependent term and the bias are constant in j, so they cancel in
softmax over j:
    softmax_j(score[b,i,:]) = softmax(key @ wk)          (same for every b,i)
    out[b,i,:]              = softmax(key @ wk) @ value  (a single 3-vector)

Sharding: data-parallel over batch B=32 -> 4 batches per core. key/value/W
are replicated; every core computes the (identical) 3-vector and writes its
own (4,512,3) output shard.

Implementation: raw Bacc (no TileContext) with manual semaphores on three
engines - sync issues all DMAs (HWDGE), vector does the elementwise work and
reductions, scalar does the exp. Tile's scheduling scaffolding (event-sem
vector clocks, per-engine drains, exit barrier butterfly) costs ~18us on a
kernel whose real work is ~3us, so it is written out by hand instead.

Cross-partition tricks (validated on HW):
  - reduce across partitions: DMA-collapse pcat (128,4) -> sfold (1,512),
    then a strided free-axis reduce on one partition.
  - broadcast result to the (2048,3) output: bounce v* (1,48) through a
    DRAM scratch, then one DRAM->DRAM DMA with a 0-step partition source
    (SBUF sources cannot have 0-step partition dims; DRAM sources can).
"""

import sys
import types

import numpy as np

import concourse.bacc as bacc
import concourse.bass as bass
from concourse import mybir
from concourse.bass_utils import run_bass_kernel_spmd


def _install_ntff_shim():
    """bass_utils' trace path imports antenv.axon_hooks, which some images
    lack. Provide it, backed by trn_boot's ctypes NTFF hook when available;
    returning None makes bass_utils skip tracing instead of crashing."""
    if "antenv.axon_hooks" in sys.modules:
        return
    try:
        import antenv

        m = types.ModuleType("antenv.axon_hooks")
        _state = {"hook": None, "tried": False}

        def set_axon_ntff_profile_hook(h):
            _state["hook"] = h
            _state["tried"] = True

        def get_axon_ntff_profile_hook():
            if not _state["tried"]:
                _state["tried"] = True
                try:
                    from trn_agent_boot.trn_boot import _ntff_profile_via_ctypes

                    _state["hook"] = _ntff_profile_via_ctypes(
                        "/opt/axon/libaxon_pjrt.so"
                    )
                except Exception:
                    _state["hook"] = None
            return _state["hook"]

        m.set_axon_ntff_profile_hook = set_axon_ntff_profile_hook
        m.get_axon_ntff_profile_hook = get_axon_ntff_profile_hook
        sys.modules["antenv.axon_hooks"] = m
        antenv.axon_hooks = m
    except Exception:
        pass


_install_ntff_shim()

N_CORES = 8
B, S1, S2, D = 32, 512, 2048, 3
P, NF = 128, 16  # 2048 = 128 partitions x 16 free
ROWS_PER_CORE = (B // N_CORES) * S1  # 2048 rows of (3,) per core

# Exposed for the test harness: the BassKernelResults of the last run
# (carries exec_time_ns when BASS_TRACE=1).
last_result = None

_nc_cache = None


def _strip_framework_overhead(nc):
    """Remove Bass' hardcoded framework scaffolding: the const-AP memsets
    (nothing here reads them) and the entry/exit all-engine barriers
    (Drain + barrier_* EventSemaphores). All ordering in this kernel is
    carried by its own semaphores; the runtime resets semaphores per
    execution, so no cross-engine barrier is needed at entry or exit."""

    def is_barrier_sync(si):
        if si is None:
            return False
        refs = [w.ant_name for w in (si.on_wait or [])] + [
            u.ant_name for u in (si.on_update or [])
        ]
        return bool(refs) and all(r.startswith("barrier_") for r in refs)

    for f in nc.m.functions:
        for bb in f.blocks:
            kept = []
            for inst in bb.instructions:
                t = type(inst).__name__
                if t == "InstDrain":
                    continue
                if (
                    t == "InstMemset"
                    and inst.outs
                    and getattr(inst.outs[0], "memref", "").startswith("const-")
                ):
                    continue
                if t == "InstEventSemaphore" and is_barrier_sync(inst.sync_info):
                    continue
                kept.append(inst)
            if len(kept) != len(bb.instructions):
                bb.instructions[:] = kept


def _build():
    nc = bacc.Bacc(target_bir_lowering=False, debug=False)
    f32 = mybir.dt.float32
    key_t = nc.dram_tensor("key", [S2, D], f32, kind="ExternalInput")
    val_t = nc.dram_tensor("value", [S2, D], f32, kind="ExternalInput")
    w_t = nc.dram_tensor("W", [1, 6], f32, kind="ExternalInput")
    out_t = nc.dram_tensor("out", [ROWS_PER_CORE, D], f32, kind="ExternalOutput")

    from contextlib import ExitStack

    # 32 partitions x 64 rows: 4x fewer DMA descriptors per transfer than a
    # 128-partition layout; DVE/ACT op time is overhead-dominated either way.
    Q, QF = 32, 64  # 2048 = 32 * 64

    with ExitStack() as ctx:
        ec = ctx.enter_context
        kv = ec(nc.sbuf_tensor("kv", [Q, QF, D], f32))
        vv = ec(nc.sbuf_tensor("vv", [Q, QF, D], f32))
        wb = ec(nc.sbuf_tensor("wb", [Q, 6], f32))
        t0 = ec(nc.sbuf_tensor("t0", [Q, QF], f32))
        t1 = ec(nc.sbuf_tensor("t1", [Q, QF], f32))
        sk = ec(nc.sbuf_tensor("sk", [Q, QF], f32))
        e = ec(nc.sbuf_tensor("e", [Q, QF], f32))
        bz = ec(nc.sbuf_tensor("bz", [Q, 1], f32))
        mall = ec(nc.sbuf_tensor("mall", [Q, QF, D], f32))
        pcat = ec(nc.sbuf_tensor("pcat", [Q, 4], f32))
        ones = ec(nc.sbuf_tensor("ones", [Q, Q], f32))
        rz = ec(nc.sbuf_tensor("rz", [Q, 1], f32))
        ot = ec(nc.sbuf_tensor("ot", [Q, QF, D], f32))
        redp = ec(nc.psum_tensor("redp", [Q, 4], f32))
        dW = ec(nc.semaphore("dW"))      # W broadcast load
        dK = ec(nc.semaphore("dK"))      # key load
        dV = ec(nc.semaphore("dV"))      # value load
        dO = ec(nc.semaphore("dO"))      # final store
        vp = ec(nc.semaphore("vp"))      # vector same-engine pipeline sem
        s_sk = ec(nc.semaphore("s_sk"))  # vector -> scalar: sk ready
        s_e = ec(nc.semaphore("s_e"))    # scalar -> vector: e ready
        s_pc = ec(nc.semaphore("s_pc"))  # vector -> tensor: pcat ready (x2)
        s_mm = ec(nc.semaphore("s_mm"))  # tensor -> vector: reduction done
        s_ot = ec(nc.semaphore("s_ot"))  # vector -> scalar: out tile ready

        # No nc.Block(): straight-line single-bb program, engines' streams
        # interleave by engine tag; ordering is purely semaphore-driven.
        # Only three engines carry work: ACT (HWDGE DMAs + exp), DVE, PE.

        # --- scalar (ACT) queue: all loads (HWDGE), key first (it gates sk)
        nc.scalar.dma_start(
            out=kv[:, :, :], in_=key_t[:, :].rearrange("(p n) d -> p n d", p=Q)
        ).then_inc(dK, 16)
        nc.scalar.dma_start(out=wb[:, :], in_=w_t[:, :].to_broadcast([Q, 6])).then_inc(
            dW, 16
        )
        nc.scalar.dma_start(
            out=vv[:, :, :], in_=val_t[:, :].rearrange("(p n) d -> p n d", p=Q)
        ).then_inc(dV, 16)

        # --- vector: sk[p,n] = sum_d key[p,n,d] * wk[d]  (wk = W[0, 3:6]) ---
        nc.vector.memset(bz[:, :], 0.0)
        nc.vector.memset(ones[:, :], 1.0)
        nc.vector.wait_ge(dK, 16)
        nc.vector.wait_ge(dW, 16)
        nc.vector.tensor_scalar_mul(t0[:, :], kv[:, :, 0], wb[:, 3:4]).then_inc(vp)
        nc.vector.wait_ge(vp, 1)
        nc.vector.scalar_tensor_tensor(
            t1[:, :], kv[:, :, 1], wb[:, 4:5], t0[:, :],
            op0=mybir.AluOpType.mult, op1=mybir.AluOpType.add,
        ).then_inc(vp)
        nc.vector.wait_ge(vp, 2)
        nc.vector.scalar_tensor_tensor(
            sk[:, :], kv[:, :, 2], wb[:, 5:6], t1[:, :],
            op0=mybir.AluOpType.mult, op1=mybir.AluOpType.add,
        ).then_inc(s_sk, 1)

        # --- scalar: e = exp(sk) ---
        nc.scalar.wait_ge(s_sk, 1)
        nc.scalar.activation(
            e[:, :], sk[:, :], mybir.ActivationFunctionType.Exp, bias=bz[:, 0:1]
        ).then_inc(s_e, 1)

        # --- vector: pcat[:,0:3] = per-partition sum_n e*value_d, [3] = sum e
        nc.vector.wait_ge(s_e, 1)
        nc.vector.wait_ge(dV, 16)
        e_b = bass.AP(tensor=e.ap().tensor, offset=0, ap=[[QF, Q], [1, QF], [0, D]])
        nc.vector.tensor_mul(mall[:, :, :], e_b, vv[:, :, :]).then_inc(vp)
        nc.vector.wait_ge(vp, 3)
        nc.vector.reduce_sum(
            pcat[:, 0:3],
            mall.ap().rearrange("p n d -> p d n"),
            axis=mybir.AxisListType.X,
        ).then_inc(s_pc, 1)
        nc.vector.reduce_sum(
            pcat[:, 3:4], e[:, :], axis=mybir.AxisListType.X
        ).then_inc(s_pc, 1)

        # --- tensor: ones(32,32).T @ pcat(32,4) reduces across partitions
        # AND broadcasts the 4 column sums to all 32 partitions in one op ---
        nc.tensor.wait_ge(s_pc, 2)
        nc.tensor.matmul(
            redp.ap()[:, :], ones[:, :], pcat[:, :], start=True, stop=True
        ).then_inc(s_mm, 1)

        # --- vector: v* = num/Z straight out of PSUM, broadcast x64 into the
        # out tile via a 0-step middle dim on the input AP ---
        nc.vector.wait_ge(s_mm, 1)
        nc.vector.reciprocal(rz[:, :], redp.ap()[:, 3:4]).then_inc(vp)
        nc.vector.wait_ge(vp, 4)
        num_b = bass.AP(
            tensor=redp.ap().tensor, offset=0, ap=[[4, Q], [0, QF], [1, D]]
        )
        nc.vector.tensor_scalar_mul(ot[:, :, :], num_b, rz[:, 0:1]).then_inc(s_ot, 1)

        # --- scalar: store the full shard ---
        nc.scalar.wait_ge(s_ot, 1)
        nc.scalar.dma_start(
            out=out_t[:, :].rearrange("(p n) d -> p n d", p=Q), in_=ot[:, :, :]
        ).then_inc(dO, 16)
        nc.scalar.wait_ge(dO, 16)

    _strip_framework_overhead(nc)
    nc.compile()
    return nc


def kernel(x, key, value, W, b):
    global last_result, _nc_cache
    key = np.ascontiguousarray(np.asarray(key, dtype=np.float32))
    value = np.ascontiguousarray(np.asarray(value, dtype=np.float32))
    W = np.ascontiguousarray(np.asarray(W, dtype=np.float32))
    if _nc_cache is None:
        _nc_cache = _build()
    in_maps = [
        {"key": key, "value": value, "W": W} for _ in range(N_CORES)
    ]
    res = run_bass_kernel_spmd(_nc_cache, in_maps, core_ids=list(range(N_CORES)))
    last_result = res
    out = np.concatenate([r["out"] for r in res.results], axis=0)
    return out.reshape(B, S1, D)
